# revision 25
# baseline (speedup 1.0000x reference)
"""Differentiable 2D log-chroma histogram on 8 Trainium2 NeuronCores.

Problem: img [4, 3, 384, 512] f32 -> out [4, 64, 64] f32 where
  u = ln(g+eps) - ln(r+eps), v = ln(g+eps) - ln(b+eps)
  Iy = sqrt(r^2+g^2+b^2) * (r+g+b > eps)
  N[b,j,i] = sum_p Iy * (0<|v - A_v[j]|<=eps_bin) * (0<|u - A_u[i]|<=eps_bin)
  out = sqrt((N+1e-8) / (sum(N+1e-8)+1e-8))

Device algorithm (per core; batch b = core//2, height-half = core%2):
  Each pixel lands in exactly 2 consecutive u-bins {k, k+1} (k = floor((u-LO)/eps))
  and 2 consecutive v-bins, so the double-hot histogram N equals a 2x2 box-sum of
  the single-hot histogram H[j', i'] (j' = k_v+1, i' = k_u+1; width 66 = 65 live
  + 1 dead column; out-of-range indices match no one-hot column and drop out).

  The DVE is the critical path (tensor_tensor is capped at 2 elem/cyc packed),
  so the v-side weighted one-hot wv = iy*onehot66(jv) is built FACTORED:
  jv = 6a + b, wv[p, 6a+b] = onehot11(a)[a] * (iy*onehot6(b))[b].  Per pixel
  that is 22 + 6 + 6 + 66 = 100 mask elements instead of 66 + 66 + 66 = 198
  for the direct {eq, eq, mult}.  The u-side one-hot stays direct (66).  All
  index/weight operands are stored as bf16 *pairs* (each value duplicated in
  adjacent columns) so broadcast access patterns keep innermost step=1 and the
  DVE runs in 2x_1P packed mode; onehot11 is built pair-duplicated (host iota
  0,0,1,1,..) so the 66-wide combine can broadcast it over the b-dim with
  innermost step=1.  Indices use a 1.5*2^23 magic-round bias: x+bias stays in
  [2^23, 2^24) where the f32 grid is uniformly 1.0 (with 2^23, values just
  below the bias round on a 0.5 grid and e.g. the a=0 digit becomes -0.5 and
  every jv=0 pixel is dropped).  A dedicated slab-0 prep chain (separate small
  tiles for the first 64 pixel-tiles, fed by small priority DMAs) is emitted
  before the full-width prep so mask work starts ~7us earlier.  Iota constants
  come from a host-built input (gpsimd iota + its dge_drain avoided).  H is
  accumulated on the tensor engine: per 128-pixel tile, H += wv^T @ mu into
  one PSUM bank across all 768 tiles (the PE sustains ~32ns/tile when fed, far
  below DVE cadence; weight loads must stay contiguous -- a strided-LDW layout
  measured 2x slower).  Host folds H (2x2 box sum), combines core pairs,
  normalizes, sqrts.

  Engine notes from this tuning round: ScalarE ACTIVATE is 1 elem/cyc/lane
  (moving mask work there loses), GPSIMD rejects TENSOR_TENSOR in codegen,
  per-element bias tensors do not exist (no ACT-side Exp weighting), and
  >64-tile DVE ops run ~15-20% slower per element (SBUF conflicts), so 64-tile
  chunks are the sweet spot.
"""
import os

import numpy as np

import concourse.bacc as bacc
import concourse.tile as tile
from concourse import mybir
from concourse.bass_utils import run_bass_kernel_spmd

NBINS = 64
HIST_LO, HIST_HI = -2.85, 2.85
EPS_BIN = (HIST_HI - HIST_LO) / (NBINS - 1)
EPS = 1e-8
P = 128
T = 768  # 128*768 = 98304 pixels per core = half of one batch image
NB = 66  # one-hot width: k+1 in [0, 64] + 1 dead column (= 11*6)
NA = 11  # outer digit: j' = 6*a + b
NBB = 6  # inner digit
TC0 = 64  # slab-0 tile count (small, starts the pipeline early)
TCM = 64  # max tiles per later mask chunk
CHUNK_SIZES = [64] * 11 + [44, 20]
CHUNK_STARTS = [sum(CHUNK_SIZES[:i]) for i in range(len(CHUNK_SIZES))]
assert sum(CHUNK_SIZES) == T
MAGIC = 1.5 * 2.0**23  # round-to-int bias; 1.5*2^23 keeps x+MAGIC in
# [2^23, 2^24) where the f32 grid is uniformly 1.0 (at 2^23 exactly, the
# grid below is 0.5 and e.g. a=0 digits would round to -0.5 and get dropped)

f32 = mybir.dt.float32
bf16 = mybir.dt.bfloat16
Act = mybir.ActivationFunctionType
Alu = mybir.AluOpType

_cache = {}


def _build_bass():
    nc = bacc.Bacc("TRN2", target_bir_lowering=False, debug=False, num_devices=8)
    rgb = nc.declare_dram_parameter("rgb", [3, P, T], f32, isOutput=False)
    # host-built iota constants [iota66 | iota11 dup-pairs | iota6]
    cst = nc.declare_dram_parameter("cst", [P, NB + 2 * NA + NBB], bf16, isOutput=False)
    hist = nc.declare_dram_parameter("hist", [NB, NB], f32, isOutput=True)

    with tile.TileContext(nc) as tc:
        with (
            tc.tile_pool(name="const", bufs=1) as cpool,
            tc.tile_pool(name="px", bufs=1) as px,
            tc.tile_pool(name="mask", bufs=3) as mpool,
            tc.tile_pool(name="psum", bufs=1, space="PSUM") as pp,
        ):
            # -------- slab-0 inputs (one small priority DMA, lands first) --
            rgb0 = cpool.tile([P, 3 * TC0], f32, tag="rgb0")
            nc.sync.dma_start(
                rgb0[:].rearrange("p (c t) -> c p t", c=3), rgb[:, :, 0:TC0]
            )
            r0 = rgb0[:, 0:TC0]
            g0 = rgb0[:, TC0 : 2 * TC0]
            b0 = rgb0[:, 2 * TC0 :]

            iotas = cpool.tile([P, NB + 2 * NA + NBB], bf16, tag="iotas")
            nc.sync.dma_start(iotas[:], cst[:])
            iota66 = iotas[:, 0:NB]
            iota11p = iotas[:, NB : NB + 2 * NA]
            iota6 = iotas[:, NB + 2 * NA :]

            eps_bias = cpool.tile([P, 1], f32, tag="eps_bias")
            nc.vector.memset(eps_bias[:], EPS)
            negM = cpool.tile([P, 1], f32, tag="negM")
            nc.vector.memset(negM[:], -MAGIC)
            zbias = cpool.tile([P, 1], f32, tag="zbias")
            nc.vector.memset(zbias[:], 0.0)
            # tiny dummy Ln preloads the ACT table before the DMA completes
            tbl_warm = cpool.tile([P, 1], f32, tag="tbl_warm")
            nc.scalar.activation(tbl_warm[:], eps_bias[:], Act.Ln, bias=eps_bias[:])

            # ---------------- rest of the inputs (big DMAs) ----------------
            TR = T - TC0
            r = px.tile([P, TR], f32, tag="r")
            g = px.tile([P, TR], f32, tag="g")
            b = px.tile([P, TR], f32, tag="b")
            nc.sync.dma_start(r[:], rgb[0, :, TC0:T])
            nc.sync.dma_start(g[:], rgb[1, :, TC0:T])
            nc.sync.dma_start(b[:], rgb[2, :, TC0:T])

            # ---------------- prep chain (emitted for a column range) ------
            def prep(tag, rr, gg, bbt, n):
                """Emit the index/weight prep for one column range; returns
                dict of source tiles for the bf16 pair copies."""
                t = {}

                def tl(name):
                    t[name] = (cpool if n == TC0 else px).tile(
                        [P, n], f32, name=f"{tag}{name}", tag=f"{tag}{name}"
                    )
                    return t[name]

                lr, lg, lb = tl("lr"), tl("lg"), tl("lb")
                nc.scalar.activation(lr[:], rr, Act.Ln, bias=eps_bias[:])
                nc.scalar.activation(lg[:], gg, Act.Ln, bias=eps_bias[:])
                nc.scalar.activation(lb[:], bbt, Act.Ln, bias=eps_bias[:])
                u, v = tl("u"), tl("v")
                nc.vector.tensor_tensor(u[:], lg[:], lr[:], op=Alu.subtract)
                nc.vector.tensor_tensor(v[:], lg[:], lb[:], op=Alu.subtract)
                def affine(dst, srct, s0, s1):
                    nc.vector.tensor_scalar(
                        dst[:], srct[:], s0, s1, op0=Alu.mult, op1=Alu.add
                    )
                iu, jvm = tl("iu"), tl("jvm")
                affine(iu, u, 1.0 / EPS_BIN, 0.5 - HIST_LO / EPS_BIN + MAGIC)
                affine(jvm, v, -1.0 / EPS_BIN, 0.5 + HIST_HI / EPS_BIN + MAGIC)
                jvs = tl("jvs")
                affine(jvs, jvm, 1.0, -MAGIC)
                a1 = tl("a1")
                affine(a1, jvs, 1.0 / 6.0, -2.5 / 6.0)
                am = tl("am")
                affine(am, a1, 1.0, MAGIC)
                asm = tl("asm")
                affine(asm, am, 1.0, -MAGIC)
                bsm = tl("bsm")
                nc.vector.scalar_tensor_tensor(
                    bsm[:], asm[:], -6.0, jvs[:], op0=Alu.mult, op1=Alu.add
                )
                # Iy^2 via bf16 squares (halves the cost of the adds; iy only
                # ever enters the matmul in bf16 anyway)
                r2 = (cpool if n == TC0 else px).tile(
                    [P, n], bf16, name=f"{tag}r2", tag=f"{tag}r2"
                )
                g2 = (cpool if n == TC0 else px).tile(
                    [P, n], bf16, name=f"{tag}g2", tag=f"{tag}g2"
                )
                b2 = (cpool if n == TC0 else px).tile(
                    [P, n], bf16, name=f"{tag}b2", tag=f"{tag}b2"
                )
                nc.scalar.activation(r2[:], rr, Act.Square)
                nc.scalar.activation(g2[:], gg, Act.Square)
                nc.scalar.activation(b2[:], bbt, Act.Square)
                ss = (cpool if n == TC0 else px).tile(
                    [P, n], bf16, name=f"{tag}ss", tag=f"{tag}ss"
                )
                nc.vector.tensor_tensor(ss[:], r2[:], g2[:], op=Alu.add)
                nc.vector.tensor_tensor(ss[:], ss[:], b2[:], op=Alu.add)
                t["ss"] = ss
                return t

            def pair_op(dst_ap, src_ap, bias, act, n):
                nc.scalar.activation(
                    dst_ap.rearrange("p (t two) -> p two t", two=2),
                    src_ap.unsqueeze(1).to_broadcast([P, 2, n]),
                    act,
                    bias=bias,
                )

            # slab-0 chain: prep + pairs (small tiles; chunk 0 reads these)
            s0 = prep("s0", r0, g0, b0, TC0)
            iu_p0 = cpool.tile([P, 2 * TC0], bf16, tag="iu_p0")
            a_p0 = cpool.tile([P, 2 * TC0], bf16, tag="a_p0")
            b_p0 = cpool.tile([P, 2 * TC0], bf16, tag="b_p0")
            iy_p0 = cpool.tile([P, 2 * TC0], bf16, tag="iy_p0")
            pair_op(iu_p0[:], s0["iu"][:], negM[:], Act.Identity, TC0)
            pair_op(a_p0[:], s0["am"][:], negM[:], Act.Identity, TC0)
            pair_op(b_p0[:], s0["bsm"][:], zbias[:], Act.Identity, TC0)
            pair_op(iy_p0[:], s0["ss"][:], zbias[:], Act.Sqrt, TC0)

            pairs_0 = {"iu": iu_p0, "a": a_p0, "b": b_p0, "iy": iy_p0}
            # full pair tiles cover tiles [TC0, T) at offset 2*TC0
            iu_p = px.tile([P, 2 * T], bf16, tag="iu_p")
            a_p = px.tile([P, 2 * T], bf16, tag="a_p")
            b_p = px.tile([P, 2 * T], bf16, tag="b_p")
            iy_p = px.tile([P, 2 * T], bf16, tag="iy_p")
            pairs_full = {"iu": iu_p, "a": a_p, "b": b_p, "iy": iy_p}

            def pair_bcast(key, c, inner):
                st, sz = CHUNK_STARTS[c], CHUNK_SIZES[c]
                if c == 0:
                    sl = pairs_0[key][:]
                else:
                    sl = pairs_full[key][:, st * 2 : (st + sz) * 2]
                return (
                    sl.rearrange("p (t two) -> p t two", two=2)
                    .unsqueeze(2)
                    .to_broadcast([P, sz, inner, 2])
                )

            def iota_bcast(tl, sz, inner):
                return (
                    tl.rearrange("p (h two) -> p h two", two=2)
                    .unsqueeze(1)
                    .to_broadcast([P, sz, inner, 2])
                )

            hp = pp.tile([NB, NB], f32, tag="hp")

            def emit_chunk(c):
                cst_, csz = CHUNK_STARTS[c], CHUNK_SIZES[c]
                mu = mpool.tile([P, TCM * NB], bf16, tag="mu")
                da = mpool.tile([P, TCM * 2 * NA], bf16, tag="da")
                wb = mpool.tile([P, TCM * NBB], bf16, tag="wb")
                wv = mpool.tile([P, TCM * NB], bf16, tag="wv")
                mu4 = mu[:, 0 : csz * NB].rearrange(
                    "p (t h two) -> p t h two", h=NB // 2, two=2
                )
                da4 = da[:, 0 : csz * 2 * NA].rearrange(
                    "p (t k two) -> p t k two", k=NA, two=2
                )
                wb4 = wb[:, 0 : csz * NBB].rearrange(
                    "p (t h two) -> p t h two", h=NBB // 2, two=2
                )
                nc.vector.tensor_tensor(
                    mu4, pair_bcast("iu", c, NB // 2),
                    iota_bcast(iota66, csz, NB // 2), op=Alu.is_equal,
                )
                nc.vector.tensor_tensor(
                    da4, pair_bcast("a", c, NA), iota_bcast(iota11p, csz, NA),
                    op=Alu.is_equal,
                )
                nc.vector.tensor_tensor(
                    wb4, pair_bcast("b", c, NBB // 2),
                    iota_bcast(iota6, csz, NBB // 2), op=Alu.is_equal,
                )
                nc.vector.tensor_tensor(
                    wb4, wb4, pair_bcast("iy", c, NBB // 2), op=Alu.mult
                )
                # wv[p, t, a, h, two] = da[p, t, a(dup-pair)] * wb[p, t, (h,two)]
                da_e = (
                    da[:, 0 : csz * 2 * NA]
                    .rearrange("p (t a two) -> p t a two", a=NA, two=2)
                    .unsqueeze(3)
                    .to_broadcast([P, csz, NA, NBB // 2, 2])
                )
                wb_e = (
                    wb[:, 0 : csz * NBB]
                    .rearrange("p (t h two) -> p t h two", h=NBB // 2, two=2)
                    .unsqueeze(2)
                    .to_broadcast([P, csz, NA, NBB // 2, 2])
                )
                wv5 = wv[:, 0 : csz * NB].rearrange(
                    "p (t a h two) -> p t a h two", a=NA, h=NBB // 2, two=2
                )
                nc.vector.tensor_tensor(wv5, da_e, wb_e, op=Alu.mult)
                for t in range(csz):
                    gt = cst_ + t
                    nc.tensor.matmul(
                        hp[:],
                        lhsT=wv[:, t * NB : (t + 1) * NB],
                        rhs=mu[:, t * NB : (t + 1) * NB],
                        start=(gt == 0),
                        stop=(gt == T - 1),
                    )

            # chunk 0 first (depends only on the slab chain)
            emit_chunk(0)

            # full-width prep + pairs (tiles [TC0, T))
            fp = prep("f", r[:], g[:], b[:], TR)
            pair_op(iu_p[:, 2 * TC0 :], fp["iu"][:], negM[:], Act.Identity, TR)
            pair_op(a_p[:, 2 * TC0 :], fp["am"][:], negM[:], Act.Identity, TR)
            pair_op(b_p[:, 2 * TC0 :], fp["bsm"][:], zbias[:], Act.Identity, TR)
            pair_op(iy_p[:, 2 * TC0 :], fp["ss"][:], zbias[:], Act.Sqrt, TR)

            for c in range(1, len(CHUNK_SIZES)):
                emit_chunk(c)

            hs = cpool.tile([NB, NB], f32, tag="hs")
            nc.scalar.activation(hs[:], hp[:], Act.Copy)
            nc.sync.dma_start(hist[:], hs[:])
    nc.compile()
    return nc


def kernel(img: np.ndarray) -> np.ndarray:
    B, C, H, W_ = img.shape
    assert (B, C, H, W_) == (4, 3, 384, 512)
    img = np.ascontiguousarray(np.asarray(img, dtype=np.float32))

    if "nc" not in _cache:
        _cache["nc"] = _build_bass()
    nc = _cache["nc"]

    if "cst" not in _cache:
        import ml_dtypes

        row = np.concatenate(
            [
                np.arange(NB),
                np.repeat(np.arange(NA), 2),
                np.arange(NBB),
            ]
        ).astype(ml_dtypes.bfloat16)
        _cache["cst"] = np.ascontiguousarray(np.broadcast_to(row, (P, row.size)))
    cst = _cache["cst"]

    in_maps = []
    for core in range(8):
        bb, half = divmod(core, 2)
        shard = img[bb, :, half * 192 : (half + 1) * 192, :].reshape(3, P, T)
        in_maps.append({"rgb": np.ascontiguousarray(shard), "cst": cst})

    trace = bool(int(os.environ.get("HIST_TRACE", "0")))
    res = run_bass_kernel_spmd(nc, in_maps, list(range(8)), trace=trace)
    if trace:
        print(f"HW exec time: {res.exec_time_ns} ns")
        _cache["exec_time_ns"] = res.exec_time_ns

    out = np.empty((4, NBINS, NBINS), dtype=np.float32)
    for bb in range(4):
        h = res.results[2 * bb]["hist"].astype(np.float64) + res.results[
            2 * bb + 1
        ]["hist"].astype(np.float64)
        n = (
            h[0:64, 0:64]
            + h[0:64, 1:65]
            + h[1:65, 0:64]
            + h[1:65, 1:65]
        ) + 1e-8
        norm = n.sum() + 1e-8
        out[bb] = np.sqrt(n / norm).astype(np.float32)
    return out


# revision 26
# speedup vs baseline: 1.0378x; 1.0378x over previous
"""Differentiable 2D log-chroma histogram on 8 Trainium2 NeuronCores.

Problem: img [4, 3, 384, 512] f32 -> out [4, 64, 64] f32 where
  u = ln(g+eps) - ln(r+eps), v = ln(g+eps) - ln(b+eps)
  Iy = sqrt(r^2+g^2+b^2) * (r+g+b > eps)
  N[b,j,i] = sum_p Iy * (0<|v - A_v[j]|<=eps_bin) * (0<|u - A_u[i]|<=eps_bin)
  out = sqrt((N+1e-8) / (sum(N+1e-8)+1e-8))

Device algorithm (per core; batch b = core//2, height-half = core%2):
  Each pixel lands in exactly 2 consecutive u-bins {k, k+1} (k = floor((u-LO)/eps))
  and 2 consecutive v-bins, so the double-hot histogram N equals a 2x2 box-sum of
  the single-hot histogram H[j', i'] (j' = k_v+1, i' = k_u+1; width 66 = 65 live
  + 1 dead column; out-of-range indices match no one-hot column and drop out).

  The DVE is the critical path (tensor_tensor is capped at 2 elem/cyc packed),
  so the v-side weighted one-hot wv = iy*onehot66(jv) is built FACTORED:
  jv = 6a + b, wv[p, 6a+b] = onehot11(a)[a] * (iy*onehot6(b))[b].  Per pixel
  that is 22 + 6 + 6 + 66 = 100 mask elements instead of 66 + 66 + 66 = 198
  for the direct {eq, eq, mult}.  The u-side one-hot stays direct (66).  All
  index/weight operands are stored as bf16 *pairs* (each value duplicated in
  adjacent columns) so broadcast access patterns keep innermost step=1 and the
  DVE runs in 2x_1P packed mode; onehot11 is built pair-duplicated (host iota
  0,0,1,1,..) so the 66-wide combine can broadcast it over the b-dim with
  innermost step=1.  Indices use a 1.5*2^23 magic-round bias: x+bias stays in
  [2^23, 2^24) where the f32 grid is uniformly 1.0 (with 2^23, values just
  below the bias round on a 0.5 grid and e.g. the a=0 digit becomes -0.5 and
  every jv=0 pixel is dropped).  A dedicated slab-0 prep chain (separate small
  tiles for the first 64 pixel-tiles, fed by small priority DMAs) is emitted
  before the full-width prep so mask work starts ~7us earlier.  Iota constants
  come from a host-built input (gpsimd iota + its dge_drain avoided).  H is
  accumulated on the tensor engine: per 128-pixel tile, H += wv^T @ mu into
  one PSUM bank across all 768 tiles (the PE sustains ~32ns/tile when fed, far
  below DVE cadence; weight loads must stay contiguous -- a strided-LDW layout
  measured 2x slower).  Host folds H (2x2 box sum), combines core pairs,
  normalizes, sqrts.

  Engine notes from this tuning round: ScalarE ACTIVATE is 1 elem/cyc/lane
  (moving mask work there loses), GPSIMD rejects TENSOR_TENSOR in codegen,
  per-element bias tensors do not exist (no ACT-side Exp weighting), and
  >64-tile DVE ops run ~15-20% slower per element (SBUF conflicts), so 64-tile
  chunks are the sweet spot.
"""
import os

import numpy as np

import concourse.bacc as bacc
import concourse.tile as tile
from concourse import mybir
from concourse.bass_utils import run_bass_kernel_spmd

NBINS = 64
HIST_LO, HIST_HI = -2.85, 2.85
EPS_BIN = (HIST_HI - HIST_LO) / (NBINS - 1)
EPS = 1e-8
P = 128
T = 768  # 128*768 = 98304 pixels per core = half of one batch image
NB = 66  # one-hot width: k+1 in [0, 64] + 1 dead column (= 11*6)
NA = 11  # outer digit: j' = 6*a + b
NBB = 6  # inner digit
TC0 = 64  # slab-0 tile count (small, starts the pipeline early)
TCM = 64  # max tiles per later mask chunk
CHUNK_SIZES = [64] * 11 + [44, 20]
CHUNK_STARTS = [sum(CHUNK_SIZES[:i]) for i in range(len(CHUNK_SIZES))]
assert sum(CHUNK_SIZES) == T
MAGIC = 1.5 * 2.0**23  # round-to-int bias; 1.5*2^23 keeps x+MAGIC in
# [2^23, 2^24) where the f32 grid is uniformly 1.0 (at 2^23 exactly, the
# grid below is 0.5 and e.g. a=0 digits would round to -0.5 and get dropped)

f32 = mybir.dt.float32
bf16 = mybir.dt.bfloat16
Act = mybir.ActivationFunctionType
Alu = mybir.AluOpType

_cache = {}


def _build_bass():
    nc = bacc.Bacc("TRN2", target_bir_lowering=False, debug=False, num_devices=8)
    rgb = nc.declare_dram_parameter("rgb", [3, P, T], f32, isOutput=False)
    # host-built iota constants [iota66 | iota11 dup-pairs | iota6]
    cst = nc.declare_dram_parameter("cst", [P, NB + 2 * NA + NBB], bf16, isOutput=False)
    hist = nc.declare_dram_parameter("hist", [NB, NB], f32, isOutput=True)

    with tile.TileContext(nc) as tc:
        with (
            tc.tile_pool(name="const", bufs=1) as cpool,
            tc.tile_pool(name="px", bufs=1) as px,
            tc.tile_pool(name="mask", bufs=3) as mpool,
            tc.tile_pool(name="psum", bufs=1, space="PSUM") as pp,
        ):
            # ---------------- slab-0 inputs (tiny DMAs, land first) --------
            r0 = cpool.tile([P, TC0], f32, tag="r0")
            g0 = cpool.tile([P, TC0], f32, tag="g0")
            b0 = cpool.tile([P, TC0], f32, tag="b0")
            nc.sync.dma_start(r0[:], rgb[0, :, 0:TC0])
            nc.sync.dma_start(g0[:], rgb[1, :, 0:TC0])
            nc.sync.dma_start(b0[:], rgb[2, :, 0:TC0])

            iotas = cpool.tile([P, NB + 2 * NA + NBB], bf16, tag="iotas")
            nc.sync.dma_start(iotas[:], cst[:])
            iota66 = iotas[:, 0:NB]
            iota11p = iotas[:, NB : NB + 2 * NA]
            iota6 = iotas[:, NB + 2 * NA :]

            eps_bias = cpool.tile([P, 1], f32, tag="eps_bias")
            nc.vector.memset(eps_bias[:], EPS)
            negM = cpool.tile([P, 1], f32, tag="negM")
            nc.vector.memset(negM[:], -MAGIC)
            zbias = cpool.tile([P, 1], f32, tag="zbias")
            nc.vector.memset(zbias[:], 0.0)
            # tiny dummy Ln preloads the ACT table before the DMA completes
            tbl_warm = cpool.tile([P, 1], f32, tag="tbl_warm")
            nc.scalar.activation(tbl_warm[:], eps_bias[:], Act.Ln, bias=eps_bias[:])

            # ---------------- rest of the inputs (big DMAs) ----------------
            TR = T - TC0
            r = px.tile([P, TR], f32, tag="r")
            g = px.tile([P, TR], f32, tag="g")
            b = px.tile([P, TR], f32, tag="b")
            nc.sync.dma_start(r[:], rgb[0, :, TC0:T])
            nc.sync.dma_start(g[:], rgb[1, :, TC0:T])
            nc.sync.dma_start(b[:], rgb[2, :, TC0:T])

            # ---------------- prep chain (emitted for a column range) ------
            def prep(tag, rr, gg, bbt, n):
                """Emit the index/weight prep for one column range; returns
                dict of source tiles for the bf16 pair copies."""
                t = {}

                def tl(name):
                    t[name] = (cpool if n == TC0 else px).tile(
                        [P, n], f32, name=f"{tag}{name}", tag=f"{tag}{name}"
                    )
                    return t[name]

                lr, lg, lb = tl("lr"), tl("lg"), tl("lb")
                nc.scalar.activation(lr[:], rr, Act.Ln, bias=eps_bias[:])
                nc.scalar.activation(lg[:], gg, Act.Ln, bias=eps_bias[:])
                nc.scalar.activation(lb[:], bbt, Act.Ln, bias=eps_bias[:])
                u, v = tl("u"), tl("v")
                nc.vector.tensor_tensor(u[:], lg[:], lr[:], op=Alu.subtract)
                nc.vector.tensor_tensor(v[:], lg[:], lb[:], op=Alu.subtract)
                def affine(dst, srct, s0, s1):
                    nc.vector.tensor_scalar(
                        dst[:], srct[:], s0, s1, op0=Alu.mult, op1=Alu.add
                    )
                iu, jvm = tl("iu"), tl("jvm")
                affine(iu, u, 1.0 / EPS_BIN, 0.5 - HIST_LO / EPS_BIN + MAGIC)
                affine(jvm, v, -1.0 / EPS_BIN, 0.5 + HIST_HI / EPS_BIN + MAGIC)
                jvs = tl("jvs")
                affine(jvs, jvm, 1.0, -MAGIC)
                a1 = tl("a1")
                affine(a1, jvs, 1.0 / 6.0, -2.5 / 6.0)
                am = tl("am")
                affine(am, a1, 1.0, MAGIC)
                asm = tl("asm")
                affine(asm, am, 1.0, -MAGIC)
                bsm = tl("bsm")
                nc.vector.scalar_tensor_tensor(
                    bsm[:], asm[:], -6.0, jvs[:], op0=Alu.mult, op1=Alu.add
                )
                # Iy^2 via bf16 squares (halves the cost of the adds; iy only
                # ever enters the matmul in bf16 anyway)
                r2 = (cpool if n == TC0 else px).tile(
                    [P, n], bf16, name=f"{tag}r2", tag=f"{tag}r2"
                )
                g2 = (cpool if n == TC0 else px).tile(
                    [P, n], bf16, name=f"{tag}g2", tag=f"{tag}g2"
                )
                b2 = (cpool if n == TC0 else px).tile(
                    [P, n], bf16, name=f"{tag}b2", tag=f"{tag}b2"
                )
                nc.scalar.activation(r2[:], rr, Act.Square)
                nc.scalar.activation(g2[:], gg, Act.Square)
                nc.scalar.activation(b2[:], bbt, Act.Square)
                ss = (cpool if n == TC0 else px).tile(
                    [P, n], bf16, name=f"{tag}ss", tag=f"{tag}ss"
                )
                nc.vector.tensor_tensor(ss[:], r2[:], g2[:], op=Alu.add)
                nc.vector.tensor_tensor(ss[:], ss[:], b2[:], op=Alu.add)
                t["ss"] = ss
                return t

            def pair_op(dst_ap, src_ap, bias, act, n):
                nc.scalar.activation(
                    dst_ap.rearrange("p (t two) -> p two t", two=2),
                    src_ap.unsqueeze(1).to_broadcast([P, 2, n]),
                    act,
                    bias=bias,
                )

            # slab-0 chain: prep + pairs (small tiles; chunk 0 reads these)
            s0 = prep("s0", r0[:], g0[:], b0[:], TC0)
            iu_p0 = cpool.tile([P, 2 * TC0], bf16, tag="iu_p0")
            a_p0 = cpool.tile([P, 2 * TC0], bf16, tag="a_p0")
            b_p0 = cpool.tile([P, 2 * TC0], bf16, tag="b_p0")
            iy_p0 = cpool.tile([P, 2 * TC0], bf16, tag="iy_p0")
            pair_op(iu_p0[:], s0["iu"][:], negM[:], Act.Identity, TC0)
            pair_op(a_p0[:], s0["am"][:], negM[:], Act.Identity, TC0)
            pair_op(b_p0[:], s0["bsm"][:], zbias[:], Act.Identity, TC0)
            pair_op(iy_p0[:], s0["ss"][:], zbias[:], Act.Sqrt, TC0)

            pairs_0 = {"iu": iu_p0, "a": a_p0, "b": b_p0, "iy": iy_p0}
            # full pair tiles cover tiles [TC0, T) at offset 2*TC0
            iu_p = px.tile([P, 2 * T], bf16, tag="iu_p")
            a_p = px.tile([P, 2 * T], bf16, tag="a_p")
            b_p = px.tile([P, 2 * T], bf16, tag="b_p")
            iy_p = px.tile([P, 2 * T], bf16, tag="iy_p")
            pairs_full = {"iu": iu_p, "a": a_p, "b": b_p, "iy": iy_p}

            def pair_bcast(key, c, inner):
                st, sz = CHUNK_STARTS[c], CHUNK_SIZES[c]
                if c == 0:
                    sl = pairs_0[key][:]
                else:
                    sl = pairs_full[key][:, st * 2 : (st + sz) * 2]
                return (
                    sl.rearrange("p (t two) -> p t two", two=2)
                    .unsqueeze(2)
                    .to_broadcast([P, sz, inner, 2])
                )

            def iota_bcast(tl, sz, inner):
                return (
                    tl.rearrange("p (h two) -> p h two", two=2)
                    .unsqueeze(1)
                    .to_broadcast([P, sz, inner, 2])
                )

            hp = pp.tile([NB, NB], f32, tag="hp")

            def emit_chunk(c):
                cst_, csz = CHUNK_STARTS[c], CHUNK_SIZES[c]
                mu = mpool.tile([P, TCM * NB], bf16, tag="mu")
                da = mpool.tile([P, TCM * 2 * NA], bf16, tag="da")
                wb = mpool.tile([P, TCM * NBB], bf16, tag="wb")
                wv = mpool.tile([P, TCM * NB], bf16, tag="wv")
                mu4 = mu[:, 0 : csz * NB].rearrange(
                    "p (t h two) -> p t h two", h=NB // 2, two=2
                )
                da4 = da[:, 0 : csz * 2 * NA].rearrange(
                    "p (t k two) -> p t k two", k=NA, two=2
                )
                wb4 = wb[:, 0 : csz * NBB].rearrange(
                    "p (t h two) -> p t h two", h=NBB // 2, two=2
                )
                nc.vector.tensor_tensor(
                    mu4, pair_bcast("iu", c, NB // 2),
                    iota_bcast(iota66, csz, NB // 2), op=Alu.is_equal,
                )
                nc.vector.tensor_tensor(
                    da4, pair_bcast("a", c, NA), iota_bcast(iota11p, csz, NA),
                    op=Alu.is_equal,
                )
                nc.vector.tensor_tensor(
                    wb4, pair_bcast("b", c, NBB // 2),
                    iota_bcast(iota6, csz, NBB // 2), op=Alu.is_equal,
                )
                nc.vector.tensor_tensor(
                    wb4, wb4, pair_bcast("iy", c, NBB // 2), op=Alu.mult
                )
                # wv[p, t, a, h, two] = da[p, t, a(dup-pair)] * wb[p, t, (h,two)]
                da_e = (
                    da[:, 0 : csz * 2 * NA]
                    .rearrange("p (t a two) -> p t a two", a=NA, two=2)
                    .unsqueeze(3)
                    .to_broadcast([P, csz, NA, NBB // 2, 2])
                )
                wb_e = (
                    wb[:, 0 : csz * NBB]
                    .rearrange("p (t h two) -> p t h two", h=NBB // 2, two=2)
                    .unsqueeze(2)
                    .to_broadcast([P, csz, NA, NBB // 2, 2])
                )
                wv5 = wv[:, 0 : csz * NB].rearrange(
                    "p (t a h two) -> p t a h two", a=NA, h=NBB // 2, two=2
                )
                nc.vector.tensor_tensor(wv5, da_e, wb_e, op=Alu.mult)
                for t in range(csz):
                    gt = cst_ + t
                    nc.tensor.matmul(
                        hp[:],
                        lhsT=wv[:, t * NB : (t + 1) * NB],
                        rhs=mu[:, t * NB : (t + 1) * NB],
                        start=(gt == 0),
                        stop=(gt == T - 1),
                    )

            # chunk 0 first (depends only on the slab chain)
            emit_chunk(0)

            # full-width prep + pairs (tiles [TC0, T))
            fp = prep("f", r[:], g[:], b[:], TR)
            pair_op(iu_p[:, 2 * TC0 :], fp["iu"][:], negM[:], Act.Identity, TR)
            pair_op(a_p[:, 2 * TC0 :], fp["am"][:], negM[:], Act.Identity, TR)
            pair_op(b_p[:, 2 * TC0 :], fp["bsm"][:], zbias[:], Act.Identity, TR)
            pair_op(iy_p[:, 2 * TC0 :], fp["ss"][:], zbias[:], Act.Sqrt, TR)

            for c in range(1, len(CHUNK_SIZES)):
                emit_chunk(c)

            hs = cpool.tile([NB, NB], f32, tag="hs")
            nc.scalar.activation(hs[:], hp[:], Act.Copy)
            nc.sync.dma_start(hist[:], hs[:])
    nc.compile()
    return nc


def kernel(img: np.ndarray) -> np.ndarray:
    B, C, H, W_ = img.shape
    assert (B, C, H, W_) == (4, 3, 384, 512)
    img = np.ascontiguousarray(np.asarray(img, dtype=np.float32))

    if "nc" not in _cache:
        _cache["nc"] = _build_bass()
    nc = _cache["nc"]

    if "cst" not in _cache:
        import ml_dtypes

        row = np.concatenate(
            [
                np.arange(NB),
                np.repeat(np.arange(NA), 2),
                np.arange(NBB),
            ]
        ).astype(ml_dtypes.bfloat16)
        _cache["cst"] = np.ascontiguousarray(np.broadcast_to(row, (P, row.size)))
    cst = _cache["cst"]

    in_maps = []
    for core in range(8):
        bb, half = divmod(core, 2)
        shard = img[bb, :, half * 192 : (half + 1) * 192, :].reshape(3, P, T)
        in_maps.append({"rgb": np.ascontiguousarray(shard), "cst": cst})

    trace = bool(int(os.environ.get("HIST_TRACE", "0")))
    res = run_bass_kernel_spmd(nc, in_maps, list(range(8)), trace=trace)
    if trace:
        print(f"HW exec time: {res.exec_time_ns} ns")
        _cache["exec_time_ns"] = res.exec_time_ns

    out = np.empty((4, NBINS, NBINS), dtype=np.float32)
    for bb in range(4):
        h = res.results[2 * bb]["hist"].astype(np.float64) + res.results[
            2 * bb + 1
        ]["hist"].astype(np.float64)
        n = (
            h[0:64, 0:64]
            + h[0:64, 1:65]
            + h[1:65, 0:64]
            + h[1:65, 1:65]
        ) + 1e-8
        norm = n.sum() + 1e-8
        out[bb] = np.sqrt(n / norm).astype(np.float32)
    return out


# revision 27
# speedup vs baseline: 1.0557x; 1.0172x over previous
"""Differentiable 2D log-chroma histogram on 8 Trainium2 NeuronCores.

Problem: img [4, 3, 384, 512] f32 -> out [4, 64, 64] f32 where
  u = ln(g+eps) - ln(r+eps), v = ln(g+eps) - ln(b+eps)
  Iy = sqrt(r^2+g^2+b^2) * (r+g+b > eps)
  N[b,j,i] = sum_p Iy * (0<|v - A_v[j]|<=eps_bin) * (0<|u - A_u[i]|<=eps_bin)
  out = sqrt((N+1e-8) / (sum(N+1e-8)+1e-8))

Device algorithm (per core; batch b = core//2, height-half = core%2):
  Each pixel lands in exactly 2 consecutive u-bins {k, k+1} (k = floor((u-LO)/eps))
  and 2 consecutive v-bins, so the double-hot histogram N equals a 2x2 box-sum of
  the single-hot histogram H[j', i'] (j' = k_v+1, i' = k_u+1; width 66 = 65 live
  + 1 dead column; out-of-range indices match no one-hot column and drop out).

  The DVE is the critical path (tensor_tensor is capped at 2 elem/cyc packed),
  so the v-side weighted one-hot wv = iy*onehot66(jv) is built FACTORED:
  jv = 6a + b, wv[p, 6a+b] = onehot11(a)[a] * (iy*onehot6(b))[b].  Per pixel
  that is 22 + 6 + 6 + 66 = 100 mask elements instead of 66 + 66 + 66 = 198
  for the direct {eq, eq, mult}.  The u-side one-hot stays direct (66).  All
  index/weight operands are stored as bf16 *pairs* (each value duplicated in
  adjacent columns) so broadcast access patterns keep innermost step=1 and the
  DVE runs in 2x_1P packed mode; onehot11 is built pair-duplicated (host iota
  0,0,1,1,..) so the 66-wide combine can broadcast it over the b-dim with
  innermost step=1.  Indices use a 1.5*2^23 magic-round bias: x+bias stays in
  [2^23, 2^24) where the f32 grid is uniformly 1.0 (with 2^23, values just
  below the bias round on a 0.5 grid and e.g. the a=0 digit becomes -0.5 and
  every jv=0 pixel is dropped).  A dedicated slab-0 prep chain (separate small
  tiles for the first 64 pixel-tiles, fed by small priority DMAs) is emitted
  before the full-width prep so mask work starts ~7us earlier.  Iota constants
  come from a host-built input (gpsimd iota + its dge_drain avoided).  H is
  accumulated on the tensor engine: per 128-pixel tile, H += wv^T @ mu into
  one PSUM bank across all 768 tiles (the PE sustains ~32ns/tile when fed, far
  below DVE cadence; weight loads must stay contiguous -- a strided-LDW layout
  measured 2x slower).  Host folds H (2x2 box sum), combines core pairs,
  normalizes, sqrts.

  Engine notes from this tuning round: ScalarE ACTIVATE is 1 elem/cyc/lane
  (moving mask work there loses), GPSIMD rejects TENSOR_TENSOR in codegen,
  per-element bias tensors do not exist (no ACT-side Exp weighting), and
  >64-tile DVE ops run ~15-20% slower per element (SBUF conflicts), so 64-tile
  chunks are the sweet spot.
"""
import os

import numpy as np

import concourse.bacc as bacc
import concourse.tile as tile
from concourse import mybir
from concourse.bass_utils import run_bass_kernel_spmd

NBINS = 64
HIST_LO, HIST_HI = -2.85, 2.85
EPS_BIN = (HIST_HI - HIST_LO) / (NBINS - 1)
EPS = 1e-8
P = 128
T = 768  # 128*768 = 98304 pixels per core = half of one batch image
NB = 66  # one-hot width: k+1 in [0, 64] + 1 dead column (= 11*6)
NA = 11  # outer digit: j' = 6*a + b
NBB = 6  # inner digit
TC0 = 64  # slab-0 tile count (small, starts the pipeline early)
TCM = 64  # max tiles per later mask chunk
CHUNK_SIZES = [64] * 11 + [44, 20]
CHUNK_STARTS = [sum(CHUNK_SIZES[:i]) for i in range(len(CHUNK_SIZES))]
assert sum(CHUNK_SIZES) == T
MAGIC = 1.5 * 2.0**23  # round-to-int bias; 1.5*2^23 keeps x+MAGIC in
# [2^23, 2^24) where the f32 grid is uniformly 1.0 (at 2^23 exactly, the
# grid below is 0.5 and e.g. a=0 digits would round to -0.5 and get dropped)

f32 = mybir.dt.float32
bf16 = mybir.dt.bfloat16
Act = mybir.ActivationFunctionType
Alu = mybir.AluOpType

_cache = {}


def _build_bass():
    nc = bacc.Bacc("TRN2", target_bir_lowering=False, debug=False, num_devices=8)
    rgb = nc.declare_dram_parameter("rgb", [3, P, T], f32, isOutput=False)
    # host-built iota constants [iota66 | iota11 dup-pairs | iota6]
    cst = nc.declare_dram_parameter("cst", [P, NB + 2 * NA + NBB], bf16, isOutput=False)
    hist = nc.declare_dram_parameter("hist", [NB, NB], f32, isOutput=True)

    with tile.TileContext(nc) as tc:
        with (
            tc.tile_pool(name="const", bufs=1) as cpool,
            tc.tile_pool(name="px", bufs=1) as px,
            tc.tile_pool(name="mask", bufs=3) as mpool,
            tc.tile_pool(name="psum", bufs=1, space="PSUM") as pp,
        ):
            # -------- slab-0 input: ONE priority DMA [r0|g0|b0] ------------
            # (src iteration (c,p,t) rearranged to (p,c,t) to match the flat
            # [P, 3*TC0] destination; DMA issue costs ~640ns per instruction
            # on the Sync queue, so fewer DMAs = earlier slab arrival)
            rgb0 = cpool.tile([P, 3 * TC0], f32, tag="rgb0")
            nc.sync.dma_start(
                rgb0[:], rgb[:, :, 0:TC0].rearrange("c p t -> p c t")
            )

            iotas = cpool.tile([P, NB + 2 * NA + NBB], bf16, tag="iotas")
            nc.sync.dma_start(iotas[:], cst[:])
            iota66 = iotas[:, 0:NB]
            iota11p = iotas[:, NB : NB + 2 * NA]
            iota6 = iotas[:, NB + 2 * NA :]

            eps_bias = cpool.tile([P, 1], f32, tag="eps_bias")
            nc.vector.memset(eps_bias[:], EPS)
            negM = cpool.tile([P, 1], f32, tag="negM")
            nc.vector.memset(negM[:], -MAGIC)
            zbias = cpool.tile([P, 1], f32, tag="zbias")
            nc.vector.memset(zbias[:], 0.0)
            # tiny dummy Ln preloads the ACT table before the DMA completes
            tbl_warm = cpool.tile([P, 1], f32, tag="tbl_warm")
            nc.scalar.activation(tbl_warm[:], eps_bias[:], Act.Ln, bias=eps_bias[:])

            # ---------------- rest of the inputs (big DMAs) ----------------
            TR = T - TC0
            r = px.tile([P, TR], f32, tag="r")
            g = px.tile([P, TR], f32, tag="g")
            b = px.tile([P, TR], f32, tag="b")
            nc.sync.dma_start(r[:], rgb[0, :, TC0:T])
            nc.sync.dma_start(g[:], rgb[1, :, TC0:T])
            nc.sync.dma_start(b[:], rgb[2, :, TC0:T])

            # ---------------- prep chain (emitted for a column range) ------
            def prep(tag, rr, gg, bbt, n, logs=None, sqs=None):
                """Emit the index/weight prep for one column range; returns
                dict of source tiles for the bf16 pair copies."""
                t = {}

                def tl(name):
                    t[name] = (cpool if n == TC0 else px).tile(
                        [P, n], f32, name=f"{tag}{name}", tag=f"{tag}{name}"
                    )
                    return t[name]

                if logs is None:
                    lrt, lgt, lbt = tl("lr"), tl("lg"), tl("lb")
                    lr, lg, lb = lrt[:], lgt[:], lbt[:]
                    nc.scalar.activation(lr, rr, Act.Ln, bias=eps_bias[:])
                    nc.scalar.activation(lg, gg, Act.Ln, bias=eps_bias[:])
                    nc.scalar.activation(lb, bbt, Act.Ln, bias=eps_bias[:])
                else:
                    lr, lg, lb = logs
                u, v = tl("u"), tl("v")
                nc.vector.tensor_tensor(u[:], lg, lr, op=Alu.subtract)
                nc.vector.tensor_tensor(v[:], lg, lb, op=Alu.subtract)
                def affine(dst, srct, s0, s1):
                    nc.vector.tensor_scalar(
                        dst[:], srct[:], s0, s1, op0=Alu.mult, op1=Alu.add
                    )
                iu, jvm = tl("iu"), tl("jvm")
                affine(iu, u, 1.0 / EPS_BIN, 0.5 - HIST_LO / EPS_BIN + MAGIC)
                affine(jvm, v, -1.0 / EPS_BIN, 0.5 + HIST_HI / EPS_BIN + MAGIC)
                jvs = tl("jvs")
                affine(jvs, jvm, 1.0, -MAGIC)
                a1 = tl("a1")
                affine(a1, jvs, 1.0 / 6.0, -2.5 / 6.0)
                am = tl("am")
                affine(am, a1, 1.0, MAGIC)
                asm = tl("asm")
                affine(asm, am, 1.0, -MAGIC)
                bsm = tl("bsm")
                nc.vector.scalar_tensor_tensor(
                    bsm[:], asm[:], -6.0, jvs[:], op0=Alu.mult, op1=Alu.add
                )
                # Iy^2 via bf16 squares (halves the cost of the adds; iy only
                # ever enters the matmul in bf16 anyway)
                if sqs is None:
                    r2t = (cpool if n == TC0 else px).tile(
                        [P, n], bf16, name=f"{tag}r2", tag=f"{tag}r2"
                    )
                    g2t = (cpool if n == TC0 else px).tile(
                        [P, n], bf16, name=f"{tag}g2", tag=f"{tag}g2"
                    )
                    b2t = (cpool if n == TC0 else px).tile(
                        [P, n], bf16, name=f"{tag}b2", tag=f"{tag}b2"
                    )
                    nc.scalar.activation(r2t[:], rr, Act.Square)
                    nc.scalar.activation(g2t[:], gg, Act.Square)
                    nc.scalar.activation(b2t[:], bbt, Act.Square)
                    r2, g2, b2 = r2t[:], g2t[:], b2t[:]
                else:
                    r2, g2, b2 = sqs
                ss = (cpool if n == TC0 else px).tile(
                    [P, n], bf16, name=f"{tag}ss", tag=f"{tag}ss"
                )
                nc.vector.tensor_tensor(ss[:], r2, g2, op=Alu.add)
                nc.vector.tensor_tensor(ss[:], ss[:], b2, op=Alu.add)
                t["ss"] = ss
                return t

            def pair_op(dst_ap, src_ap, bias, act, n):
                nc.scalar.activation(
                    dst_ap.rearrange("p (t two) -> p two t", two=2),
                    src_ap.unsqueeze(1).to_broadcast([P, 2, n]),
                    act,
                    bias=bias,
                )

            # slab-0 chain: ONE merged Ln + Square over [r0|g0|b0], then prep
            lrgb0 = cpool.tile([P, 3 * TC0], f32, tag="lrgb0")
            nc.scalar.activation(lrgb0[:], rgb0[:], Act.Ln, bias=eps_bias[:])
            sq0 = cpool.tile([P, 3 * TC0], bf16, tag="sq0")
            nc.scalar.activation(sq0[:], rgb0[:], Act.Square)
            s0 = prep(
                "s0", None, None, None, TC0,
                logs=(
                    lrgb0[:, 0:TC0],
                    lrgb0[:, TC0 : 2 * TC0],
                    lrgb0[:, 2 * TC0 :],
                ),
                sqs=(
                    sq0[:, 0:TC0],
                    sq0[:, TC0 : 2 * TC0],
                    sq0[:, 2 * TC0 :],
                ),
            )
            iu_p0 = cpool.tile([P, 2 * TC0], bf16, tag="iu_p0")
            a_p0 = cpool.tile([P, 2 * TC0], bf16, tag="a_p0")
            b_p0 = cpool.tile([P, 2 * TC0], bf16, tag="b_p0")
            iy_p0 = cpool.tile([P, 2 * TC0], bf16, tag="iy_p0")
            pair_op(iu_p0[:], s0["iu"][:], negM[:], Act.Identity, TC0)
            pair_op(a_p0[:], s0["am"][:], negM[:], Act.Identity, TC0)
            pair_op(b_p0[:], s0["bsm"][:], zbias[:], Act.Identity, TC0)
            pair_op(iy_p0[:], s0["ss"][:], zbias[:], Act.Sqrt, TC0)

            pairs_0 = {"iu": iu_p0, "a": a_p0, "b": b_p0, "iy": iy_p0}
            # full pair tiles cover tiles [TC0, T) at offset 2*TC0
            iu_p = px.tile([P, 2 * T], bf16, tag="iu_p")
            a_p = px.tile([P, 2 * T], bf16, tag="a_p")
            b_p = px.tile([P, 2 * T], bf16, tag="b_p")
            iy_p = px.tile([P, 2 * T], bf16, tag="iy_p")
            pairs_full = {"iu": iu_p, "a": a_p, "b": b_p, "iy": iy_p}

            def pair_bcast(key, c, inner):
                st, sz = CHUNK_STARTS[c], CHUNK_SIZES[c]
                if c == 0:
                    sl = pairs_0[key][:]
                else:
                    sl = pairs_full[key][:, st * 2 : (st + sz) * 2]
                return (
                    sl.rearrange("p (t two) -> p t two", two=2)
                    .unsqueeze(2)
                    .to_broadcast([P, sz, inner, 2])
                )

            def iota_bcast(tl, sz, inner):
                return (
                    tl.rearrange("p (h two) -> p h two", two=2)
                    .unsqueeze(1)
                    .to_broadcast([P, sz, inner, 2])
                )

            hp = pp.tile([NB, NB], f32, tag="hp")

            def emit_chunk(c):
                cst_, csz = CHUNK_STARTS[c], CHUNK_SIZES[c]
                mu = mpool.tile([P, TCM * NB], bf16, tag="mu")
                da = mpool.tile([P, TCM * 2 * NA], bf16, tag="da")
                wb = mpool.tile([P, TCM * NBB], bf16, tag="wb")
                wv = mpool.tile([P, TCM * NB], bf16, tag="wv")
                mu4 = mu[:, 0 : csz * NB].rearrange(
                    "p (t h two) -> p t h two", h=NB // 2, two=2
                )
                da4 = da[:, 0 : csz * 2 * NA].rearrange(
                    "p (t k two) -> p t k two", k=NA, two=2
                )
                wb4 = wb[:, 0 : csz * NBB].rearrange(
                    "p (t h two) -> p t h two", h=NBB // 2, two=2
                )
                nc.vector.tensor_tensor(
                    mu4, pair_bcast("iu", c, NB // 2),
                    iota_bcast(iota66, csz, NB // 2), op=Alu.is_equal,
                )
                nc.vector.tensor_tensor(
                    da4, pair_bcast("a", c, NA), iota_bcast(iota11p, csz, NA),
                    op=Alu.is_equal,
                )
                nc.vector.tensor_tensor(
                    wb4, pair_bcast("b", c, NBB // 2),
                    iota_bcast(iota6, csz, NBB // 2), op=Alu.is_equal,
                )
                nc.vector.tensor_tensor(
                    wb4, wb4, pair_bcast("iy", c, NBB // 2), op=Alu.mult
                )
                # wv[p, t, a, h, two] = da[p, t, a(dup-pair)] * wb[p, t, (h,two)]
                da_e = (
                    da[:, 0 : csz * 2 * NA]
                    .rearrange("p (t a two) -> p t a two", a=NA, two=2)
                    .unsqueeze(3)
                    .to_broadcast([P, csz, NA, NBB // 2, 2])
                )
                wb_e = (
                    wb[:, 0 : csz * NBB]
                    .rearrange("p (t h two) -> p t h two", h=NBB // 2, two=2)
                    .unsqueeze(2)
                    .to_broadcast([P, csz, NA, NBB // 2, 2])
                )
                wv5 = wv[:, 0 : csz * NB].rearrange(
                    "p (t a h two) -> p t a h two", a=NA, h=NBB // 2, two=2
                )
                nc.vector.tensor_tensor(wv5, da_e, wb_e, op=Alu.mult)
                for t in range(csz):
                    gt = cst_ + t
                    nc.tensor.matmul(
                        hp[:],
                        lhsT=wv[:, t * NB : (t + 1) * NB],
                        rhs=mu[:, t * NB : (t + 1) * NB],
                        start=(gt == 0),
                        stop=(gt == T - 1),
                    )

            # chunk 0 first (depends only on the slab chain)
            emit_chunk(0)

            # full-width prep + pairs (tiles [TC0, T))
            fp = prep("f", r[:], g[:], b[:], TR)
            pair_op(iu_p[:, 2 * TC0 :], fp["iu"][:], negM[:], Act.Identity, TR)
            pair_op(a_p[:, 2 * TC0 :], fp["am"][:], negM[:], Act.Identity, TR)
            pair_op(b_p[:, 2 * TC0 :], fp["bsm"][:], zbias[:], Act.Identity, TR)
            pair_op(iy_p[:, 2 * TC0 :], fp["ss"][:], zbias[:], Act.Sqrt, TR)

            for c in range(1, len(CHUNK_SIZES)):
                emit_chunk(c)

            hs = cpool.tile([NB, NB], f32, tag="hs")
            nc.scalar.activation(hs[:], hp[:], Act.Copy)
            nc.sync.dma_start(hist[:], hs[:])
    nc.compile()
    return nc


def kernel(img: np.ndarray) -> np.ndarray:
    B, C, H, W_ = img.shape
    assert (B, C, H, W_) == (4, 3, 384, 512)
    img = np.ascontiguousarray(np.asarray(img, dtype=np.float32))

    if "nc" not in _cache:
        _cache["nc"] = _build_bass()
    nc = _cache["nc"]

    if "cst" not in _cache:
        import ml_dtypes

        row = np.concatenate(
            [
                np.arange(NB),
                np.repeat(np.arange(NA), 2),
                np.arange(NBB),
            ]
        ).astype(ml_dtypes.bfloat16)
        _cache["cst"] = np.ascontiguousarray(np.broadcast_to(row, (P, row.size)))
    cst = _cache["cst"]

    in_maps = []
    for core in range(8):
        bb, half = divmod(core, 2)
        shard = img[bb, :, half * 192 : (half + 1) * 192, :].reshape(3, P, T)
        in_maps.append({"rgb": np.ascontiguousarray(shard), "cst": cst})

    trace = bool(int(os.environ.get("HIST_TRACE", "0")))
    res = run_bass_kernel_spmd(nc, in_maps, list(range(8)), trace=trace)
    if trace:
        print(f"HW exec time: {res.exec_time_ns} ns")
        _cache["exec_time_ns"] = res.exec_time_ns

    out = np.empty((4, NBINS, NBINS), dtype=np.float32)
    for bb in range(4):
        h = res.results[2 * bb]["hist"].astype(np.float64) + res.results[
            2 * bb + 1
        ]["hist"].astype(np.float64)
        n = (
            h[0:64, 0:64]
            + h[0:64, 1:65]
            + h[1:65, 0:64]
            + h[1:65, 1:65]
        ) + 1e-8
        norm = n.sum() + 1e-8
        out[bb] = np.sqrt(n / norm).astype(np.float32)
    return out
